# revision 21
# baseline (speedup 1.0000x reference)
"""RIENet loss kernel (keypoint/KNN MSE + global-align Huber-min loss) on 8 trn2 cores.

Sharding: core ci -> (b = ci // 4, n-chunk j = ci % 4).  Each core holds the full
tgt[b] (M=8192 points) and a 2048-column chunk of src_transformed[b] (N axis).
  loss_1 (min over M per src point): complete locally per core.
  loss_2 (min over N per tgt point): per-core partial min over its chunk;
          host min-reduces the 4 chunks per batch element.

Device kernel per core (v3 — host-prepped operands, fp16 min path):
  Operand prep moved to the host: the 2-way bf16 splits of (-2*t) and s, the
  ||s||^2 split rows, and ||t||^2 are computed in numpy and DMA'd in directly
  (kills the on-device transpose/DMA preamble of v2).
  Q[m, n] = -2 t_m . s_n + ||s_n||^2 from one K=11 bf16 matmul per (m-tile,
  512-col block); the dropped tl*sl term is ~2^-17 relative, far inside the
  2e-2 tolerance.
  Per 128-row m-tile: ScalarE writes qn = fp16(Q + ||t_m||^2) (ACTIVATE
  Identity with a per-partition bias AP) into one slot of a 4-tile group
  buffer.  DVE work is batched per group of 4 m-tiles to amortize the
  ~150-cycle per-op overhead and the 1x-only tensor_reduce:
    acc4 = min(acc4, qn4)                       [128,4,2048] fp16 2x
    f1q  = min(qn4[..lo], qn4[..hi])            [128,4,1024] 2x
    f2q  = min(f1q[..lo], f1q[..hi])            [128,4,512]  2x
    rowbuf[:, 4q:4q+4] = reduce_min_X(f2q)      1x
  (tensor_tensor_reduce would fuse fold+reduce but its min/min form
  crashes the exec unit on hw; gpsimd tensor_tensor is rejected by
  walrus codegen, so no Pool-engine offload either)
  fp16 is safe: mins are order-statistics (abs err ~ val*2^-12, and loss
  sensitivity d huber/dx <= max(x, 0.1)).
  Finalize: PE-transpose acc into PSUM, one 2048-wide DVE min-reduce ->
  per-n colmin.  Tiny keypoint/KNN MSE losses run on-device on every core.
"""

import os
import numpy as np


def _ensure_path():
    try:
        import concourse  # noqa: F401
    except ImportError:
        import sys
        for p in ("/opt/trn_rl_repo", "/root/.axon_site/_ro/trn_rl_repo"):
            if os.path.isdir(p) and p not in sys.path:
                sys.path.insert(0, p)


_ensure_path()

import concourse.bass as bass  # noqa: E402
import concourse.bacc as bacc  # noqa: E402
import concourse.tile as tile  # noqa: E402
import concourse.mybir as mybir  # noqa: E402
from concourse.bass_utils import run_bass_kernel_spmd  # noqa: E402

import ml_dtypes  # noqa: E402

F32 = mybir.dt.float32
F16 = mybir.dt.float16
BF16 = mybir.dt.bfloat16
AL = mybir.AluOpType
AF = mybir.ActivationFunctionType

BF16NP = ml_dtypes.bfloat16

MARGIN = 0.1
B, KP, KNN, N, M = 2, 256, 32, 8192, 8192
NCORES = 8
NSHARDS = NCORES // B          # 4 n-chunks per batch element
CHUNK = N // NSHARDS           # 2048
NJ = CHUNK // 512              # 4 psum banks per m-tile
MI = M // 128                  # 64 m-tiles
NBLK = CHUNK // 128            # 16 column blocks for the final transpose
K11 = 11                       # 9 bf16-split product rows + 2 ||s||^2 rows
F16BIG = 65504.0               # fp16 max (acc init / reduce seed)
QG = 4                         # m-tiles per batched DVE group

_CACHE = {}


def _build():
    nc = bacc.Bacc("TRN2", target_bir_lowering=False, debug=False,
                   num_devices=NCORES)

    tA = nc.dram_tensor("tA", [K11, M], BF16, kind="ExternalInput")
    sA = nc.dram_tensor("sA", [K11, CHUNK], BF16, kind="ExternalInput")
    ntd = nc.dram_tensor("nt", [128, MI], F32, kind="ExternalInput")
    identh = nc.dram_tensor("identh", [128, 128], F16, kind="ExternalInput")
    kp_lhsT = nc.dram_tensor("kp_lhsT", [4, 2 * 3], F32, kind="ExternalInput")
    kp_rhs = nc.dram_tensor("kp_rhs", [4, 2 * KP], F32, kind="ExternalInput")
    tgt_kp = nc.dram_tensor("tgt_kp", [3, 2 * KP], F32, kind="ExternalInput")
    knn_src = nc.dram_tensor("knn_src", [128, 2 * 192], F32, kind="ExternalInput")
    knn_tgt = nc.dram_tensor("knn_tgt", [128, 2 * 192], F32, kind="ExternalInput")

    colmin_o = nc.dram_tensor("colmin", [128, NBLK], F32, kind="ExternalOutput")
    rowmin_o = nc.dram_tensor("rowmin", [128, MI], F32, kind="ExternalOutput")
    misc_o = nc.dram_tensor("misc", [128, 4], F32, kind="ExternalOutput")

    with tile.TileContext(nc) as tc:
        with (
            tc.tile_pool(name="const", bufs=1) as const,
            tc.tile_pool(name="sc", bufs=2) as sc,
        ):
            tA_sb = const.tile([K11, M], BF16)
            sA_sb = const.tile([K11, CHUNK], BF16)
            nt_sb = const.tile([128, MI], F32)
            idh = const.tile([128, 128], F16)
            acc4 = const.tile([128, QG, CHUNK], F16)
            acc = const.tile([128, CHUNK], F16)          # merged accumulator
            f1q = const.tile([128, QG, CHUNK // 2], F16)
            f2q = const.tile([128, QG, CHUNK // 4], F16)
            f3q = const.tile([128, QG, CHUNK // 8], F16)
            f4q = const.tile([128, QG, CHUNK // 16], F16)
            actwarm = const.tile([1, 1], F32)
            rowbuf = const.tile([128, MI], F32)
            colmin_sb = const.tile([128, NBLK], F32)
            misc_sb = const.tile([128, 4], F32)

            # DMA order matters for pipeline fill: the first m-tiles need
            # only sA, the head of tA, and nt — ship those first
            nc.sync.dma_start(out=sA_sb[:], in_=sA[:])
            nc.sync.dma_start(out=tA_sb[:, :1024], in_=tA[:, :1024])
            nc.sync.dma_start(out=nt_sb[:], in_=ntd[:])
            nc.sync.dma_start(out=tA_sb[:, 1024:], in_=tA[:, 1024:])
            nc.sync.dma_start(out=idh[:], in_=identh[:])
            nc.gpsimd.memset(acc4[:], F16BIG)
            nc.gpsimd.memset(misc_sb[:], 0.0)
            # absorb the one-time ACT table load while DMAs are in flight
            # (reads idh, which only needs its DMA — no engine dependency)
            nc.scalar.activation(out=actwarm[:], in_=idh[0:1, 0:1],
                                 func=AF.Identity, bias=0.0, scale=1.0)

            # ---- main loop: Q = -2 t.s + ||s||^2 per 128-row m-tile ----
            with (
                tc.tile_pool(name="psum_main", bufs=2, space="PSUM") as pm,
                tc.tile_pool(name="qpool", bufs=3) as qp,
            ):
                # prologue: first QG tiles unbatched so DVE starts after
                # tile 0 instead of tile QG-1 (cuts the pipeline-fill gap)
                for mi in range(QG):
                    pt = pm.tile([128, CHUNK], F32, tag="pt")
                    for nj in range(NJ):
                        nc.tensor.matmul(
                            pt[:, nj * 512:(nj + 1) * 512],
                            lhsT=tA_sb[:, mi * 128:(mi + 1) * 128],
                            rhs=sA_sb[:, nj * 512:(nj + 1) * 512],
                            start=True, stop=True,
                        )
                    # ScalarE writes acc4 slot mi directly (its first value),
                    # and the fold tree reads the same data back from acc4
                    nc.scalar.activation(
                        out=acc4[:, mi, :], in_=pt[:], func=AF.Identity,
                        bias=nt_sb[:, mi:mi + 1], scale=1.0)
                    nc.vector.tensor_tensor(
                        out=f1q[:, 0, :], in0=acc4[:, mi, :CHUNK // 2],
                        in1=acc4[:, mi, CHUNK // 2:], op=AL.min)
                    nc.vector.tensor_tensor(
                        out=f2q[:, 0, :], in0=f1q[:, 0, :CHUNK // 4],
                        in1=f1q[:, 0, CHUNK // 4:], op=AL.min)
                    nc.vector.tensor_reduce(
                        out=rowbuf[:, mi:mi + 1], in_=f2q[:, 0, :],
                        axis=mybir.AxisListType.X, op=AL.min)

                for qg in range(1, MI // QG):
                    qn4 = qp.tile([128, QG, CHUNK], F16, tag="qn4")
                    for i in range(QG):
                        mi = qg * QG + i
                        pt = pm.tile([128, CHUNK], F32, tag="pt")
                        for nj in range(NJ):
                            nc.tensor.matmul(
                                pt[:, nj * 512:(nj + 1) * 512],
                                lhsT=tA_sb[:, mi * 128:(mi + 1) * 128],
                                rhs=sA_sb[:, nj * 512:(nj + 1) * 512],
                                start=True, stop=True,
                            )
                        # qn = fp16(Q + ||t||^2) : PSUM -> SBUF on ScalarE
                        nc.scalar.activation(
                            out=qn4[:, i, :], in_=pt[:], func=AF.Identity,
                            bias=nt_sb[:, mi:mi + 1], scale=1.0)
                    # colmin accumulate (DVE, fp16 2x, 4 tiles at once)
                    nc.vector.tensor_tensor(
                        out=acc4[:], in0=acc4[:], in1=qn4[:], op=AL.min)
                    # rowmin: two 2x fold mins, then one 1x reduce -> 4 cols
                    nc.vector.tensor_tensor(
                        out=f1q[:], in0=qn4[:, :, :CHUNK // 2],
                        in1=qn4[:, :, CHUNK // 2:], op=AL.min)
                    nc.vector.tensor_tensor(
                        out=f2q[:], in0=f1q[:, :, :CHUNK // 4],
                        in1=f1q[:, :, CHUNK // 4:], op=AL.min)
                    nc.vector.tensor_tensor(
                        out=f3q[:], in0=f2q[:, :, :CHUNK // 8],
                        in1=f2q[:, :, CHUNK // 8:], op=AL.min)
                    nc.vector.tensor_tensor(
                        out=f4q[:], in0=f3q[:, :, :CHUNK // 16],
                        in1=f3q[:, :, CHUNK // 16:], op=AL.min)
                    nc.vector.tensor_reduce(
                        out=rowbuf[:, qg * QG:(qg + 1) * QG], in_=f4q[:],
                        axis=mybir.AxisListType.X, op=AL.min)
                # merge the 4 group accumulators into one [128, CHUNK]
                nc.vector.tensor_tensor(
                    out=acc4[:, 0, :], in0=acc4[:, 0, :], in1=acc4[:, 1, :],
                    op=AL.min)
                nc.vector.tensor_tensor(
                    out=acc4[:, 2, :], in0=acc4[:, 2, :], in1=acc4[:, 3, :],
                    op=AL.min)
                nc.vector.tensor_tensor(
                    out=acc[:], in0=acc4[:, 0, :], in1=acc4[:, 2, :],
                    op=AL.min)

            with tc.tile_pool(name="psum_fin", bufs=2, space="PSUM") as pf:
                # tiny keypoint / knn losses (both batch elements) — emitted
                # first so DVE has work while PE runs the acc transposes
                kp_l = const.tile([4, 2 * 3], F32)
                kp_r = const.tile([4, 2 * KP], F32)
                kp_t = const.tile([3, 2 * KP], F32)
                ks = const.tile([128, 2 * 192], F32)
                kt = const.tile([128, 2 * 192], F32)
                nc.sync.dma_start(out=kp_l[:], in_=kp_lhsT[:])
                nc.sync.dma_start(out=kp_r[:], in_=kp_rhs[:])
                nc.sync.dma_start(out=kp_t[:], in_=tgt_kp[:])
                nc.sync.dma_start(out=ks[:], in_=knn_src[:])
                nc.sync.dma_start(out=kt[:], in_=knn_tgt[:])
                for b in range(B):
                    pt2 = pf.tile([3, KP], F32, tag="kp")
                    nc.tensor.matmul(
                        pt2[:], lhsT=kp_l[:, b * 3:(b + 1) * 3],
                        rhs=kp_r[:, b * KP:(b + 1) * KP],
                        start=True, stop=True)
                    diff = sc.tile([3, KP], F32, tag="kdiff")
                    nc.vector.tensor_sub(diff[:], pt2[:],
                                         kp_t[:, b * KP:(b + 1) * KP])
                    nc.vector.tensor_mul(diff[:], diff[:], diff[:])
                    nc.vector.tensor_reduce(
                        out=misc_sb[0:3, b:b + 1], in_=diff[:],
                        axis=mybir.AxisListType.X, op=AL.add)
                    diff2 = sc.tile([128, 192], F32, tag="ndiff")
                    nc.vector.tensor_sub(diff2[:], ks[:, b * 192:(b + 1) * 192],
                                         kt[:, b * 192:(b + 1) * 192])
                    nc.vector.tensor_mul(diff2[:], diff2[:], diff2[:])
                    nc.vector.tensor_reduce(
                        out=misc_sb[:, 2 + b:3 + b], in_=diff2[:],
                        axis=mybir.AxisListType.X, op=AL.add)

                # partition-axis min of acc: PE transposes + one wide reduce
                tp = pf.tile([128, NBLK, 128], F16)
                for blk in range(NBLK):
                    nc.tensor.transpose(tp[:, blk, :],
                                        acc[:, blk * 128:(blk + 1) * 128],
                                        idh[:])
                nc.vector.tensor_reduce(
                    out=colmin_sb[:], in_=tp[:],
                    axis=mybir.AxisListType.X, op=AL.min)

            nc.sync.dma_start(out=colmin_o[:], in_=colmin_sb[:])
            nc.sync.dma_start(out=rowmin_o[:], in_=rowbuf[:])
            nc.sync.dma_start(out=misc_o[:], in_=misc_sb[:])

    nc.compile()
    return nc


def _get_nc():
    if "nc" not in _CACHE:
        _CACHE["nc"] = _build()
    return _CACHE["nc"]


def _prepare_in_maps(src_keypoints, tgt_keypoints, rotation_ab, translation_ab,
                     src_keypoints_knn, tgt_keypoints_knn, src_transformed, tgt):
    f = np.float32
    st = np.ascontiguousarray(np.asarray(src_transformed, dtype=f))
    tg = np.ascontiguousarray(np.asarray(tgt, dtype=f))
    skp = np.asarray(src_keypoints, dtype=f)
    tkp = np.asarray(tgt_keypoints, dtype=f)
    rot = np.asarray(rotation_ab, dtype=f)
    tra = np.asarray(translation_ab, dtype=f)
    sknn = np.asarray(src_keypoints_knn, dtype=f)
    tknn = np.asarray(tgt_keypoints_knn, dtype=f)

    identh = np.eye(128, dtype=np.float16)
    kp_lhsT = np.zeros((4, 2 * 3), dtype=f)
    kp_rhs = np.zeros((4, 2 * KP), dtype=f)
    tgt_kp = np.zeros((3, 2 * KP), dtype=f)
    knn_src = np.zeros((128, 2 * 192), dtype=f)
    knn_tgt = np.zeros((128, 2 * 192), dtype=f)
    for b in range(B):
        kp_lhsT[0:3, b * 3:(b + 1) * 3] = rot[b].T
        kp_lhsT[3, b * 3:(b + 1) * 3] = tra[b]
        kp_rhs[0:3, b * KP:(b + 1) * KP] = skp[b]
        kp_rhs[3, b * KP:(b + 1) * KP] = 1.0
        tgt_kp[:, b * KP:(b + 1) * KP] = tkp[b]
        knn_src[:, b * 192:(b + 1) * 192] = sknn[b].reshape(128, 192)
        knn_tgt[:, b * 192:(b + 1) * 192] = tknn[b].reshape(128, 192)

    # t-side (tgt, M axis): 2-way bf16 split of -2*t, plus ||t||^2 in fp32
    tAs, nts = [], []
    for b in range(B):
        t2 = (-2.0 * tg[b]).astype(f)                       # [3, M]
        th = t2.astype(BF16NP)
        tl = (t2 - th.astype(f)).astype(BF16NP)
        tA = np.empty((K11, M), dtype=BF16NP)
        tA[0:3] = th
        tA[3:6] = th
        tA[6:9] = tl
        tA[9:11] = BF16NP(1.0)
        tAs.append(tA)
        nt = (tg[b].astype(np.float64) ** 2).sum(axis=0).astype(f)   # [M]
        nts.append(np.ascontiguousarray(nt.reshape(MI, 128).T))      # [128, MI]

    shared = {
        "identh": identh, "kp_lhsT": kp_lhsT, "kp_rhs": kp_rhs,
        "tgt_kp": tgt_kp, "knn_src": knn_src, "knn_tgt": knn_tgt,
    }
    in_maps = []
    for ci in range(NCORES):
        b, j = divmod(ci, NSHARDS)
        s = st[b][:, j * CHUNK:(j + 1) * CHUNK]             # [3, CHUNK]
        sh = s.astype(BF16NP)
        sl = (s - sh.astype(f)).astype(BF16NP)
        ns64 = (s.astype(np.float64) ** 2).sum(axis=0)      # [CHUNK]
        nsh = ns64.astype(BF16NP)
        nsl = (ns64 - nsh.astype(np.float64)).astype(BF16NP)
        sA = np.empty((K11, CHUNK), dtype=BF16NP)
        sA[0:3] = sh
        sA[3:6] = sl
        sA[6:9] = sh
        sA[9] = nsh
        sA[10] = nsl
        m = dict(shared)
        m["tA"] = tAs[b]
        m["sA"] = sA
        m["nt"] = nts[b]
        in_maps.append(m)
    return in_maps


def _huber(x, c):
    return np.where(x < c, 0.5 * x * x, c * x - 0.5 * c * c)


def _postprocess(results):
    c = np.float64(MARGIN)
    loss1 = np.float64(0.0)
    loss2 = np.float64(0.0)
    for b in range(B):
        rowmins = []
        for j in range(NSHARDS):
            r = results[b * NSHARDS + j]
            colmin = np.asarray(r["colmin"], dtype=np.float64).T.ravel()
            loss1 += _huber(colmin, c).sum()
            rowmins.append(np.asarray(r["rowmin"], dtype=np.float64).T.ravel())
        rm = np.minimum.reduce(rowmins)
        loss2 += _huber(rm, c).sum()
    gal = loss1 + loss2

    misc = np.asarray(results[0]["misc"], dtype=np.float64)
    kp_loss = (misc[0:3, 0].sum() + misc[0:3, 1].sum()) / B
    knn_loss = (misc[:, 2].sum() + misc[:, 3].sum()) / (B * KNN)
    ncl = knn_loss + kp_loss
    return np.float32(ncl), np.float32(gal)


def run_device(in_maps, **kw):
    nc = _get_nc()
    return run_bass_kernel_spmd(nc, in_maps, list(range(NCORES)), **kw)


def kernel(src_keypoints, tgt_keypoints, rotation_ab, translation_ab,
           src_keypoints_knn, tgt_keypoints_knn, k, src_transformed, tgt,
           **_unused):
    in_maps = _prepare_in_maps(src_keypoints, tgt_keypoints, rotation_ab,
                               translation_ab, src_keypoints_knn,
                               tgt_keypoints_knn, src_transformed, tgt)
    res = run_device(in_maps)
    return _postprocess(res.results)


# revision 22
# speedup vs baseline: 1.0073x; 1.0073x over previous
"""RIENet loss kernel (keypoint/KNN MSE + global-align Huber-min loss) on 8 trn2 cores.

Sharding: core ci -> (b = ci // 4, n-chunk j = ci % 4).  Each core holds the full
tgt[b] (M=8192 points) and a 2048-column chunk of src_transformed[b] (N axis).
  loss_1 (min over M per src point): complete locally per core.
  loss_2 (min over N per tgt point): per-core partial min over its chunk;
          host min-reduces the 4 chunks per batch element.

Device kernel per core (v3 — host-prepped operands, fp16 min path):
  Operand prep moved to the host: the 2-way bf16 splits of (-2*t) and s, the
  ||s||^2 split rows, and ||t||^2 are computed in numpy and DMA'd in directly
  (kills the on-device transpose/DMA preamble of v2).
  Q[m, n] = -2 t_m . s_n + ||s_n||^2 from one K=11 bf16 matmul per (m-tile,
  512-col block); the dropped tl*sl term is ~2^-17 relative, far inside the
  2e-2 tolerance.
  Per 128-row m-tile: ScalarE writes qn = fp16(Q + ||t_m||^2) (ACTIVATE
  Identity with a per-partition bias AP) into one slot of a 4-tile group
  buffer.  DVE work is batched per group of 4 m-tiles to amortize the
  ~150-cycle per-op overhead and the 1x-only tensor_reduce:
    acc4 = min(acc4, qn4)                       [128,4,2048] fp16 2x
    f1q  = min(qn4[..lo], qn4[..hi])            [128,4,1024] 2x
    f2q  = min(f1q[..lo], f1q[..hi])            [128,4,512]  2x
    rowbuf[:, 4q:4q+4] = reduce_min_X(f2q)      1x
  (tensor_tensor_reduce would fuse fold+reduce but its min/min form
  crashes the exec unit on hw; gpsimd tensor_tensor is rejected by
  walrus codegen, so no Pool-engine offload either)
  fp16 is safe: mins are order-statistics (abs err ~ val*2^-12, and loss
  sensitivity d huber/dx <= max(x, 0.1)).
  Finalize: PE-transpose acc into PSUM, one 2048-wide DVE min-reduce ->
  per-n colmin.  Tiny keypoint/KNN MSE losses run on-device on every core.
"""

import os
import numpy as np


def _ensure_path():
    try:
        import concourse  # noqa: F401
    except ImportError:
        import sys
        for p in ("/opt/trn_rl_repo", "/root/.axon_site/_ro/trn_rl_repo"):
            if os.path.isdir(p) and p not in sys.path:
                sys.path.insert(0, p)


_ensure_path()

import concourse.bass as bass  # noqa: E402
import concourse.bacc as bacc  # noqa: E402
import concourse.tile as tile  # noqa: E402
import concourse.mybir as mybir  # noqa: E402
from concourse.bass_utils import run_bass_kernel_spmd  # noqa: E402

import ml_dtypes  # noqa: E402

F32 = mybir.dt.float32
F16 = mybir.dt.float16
BF16 = mybir.dt.bfloat16
AL = mybir.AluOpType
AF = mybir.ActivationFunctionType

BF16NP = ml_dtypes.bfloat16

MARGIN = 0.1
B, KP, KNN, N, M = 2, 256, 32, 8192, 8192
NCORES = 8
NSHARDS = NCORES // B          # 4 n-chunks per batch element
CHUNK = N // NSHARDS           # 2048
NJ = CHUNK // 512              # 4 psum banks per m-tile
MI = M // 128                  # 64 m-tiles
NBLK = CHUNK // 128            # 16 column blocks for the final transpose
K11 = 11                       # 9 bf16-split product rows + 2 ||s||^2 rows
F16BIG = 65504.0               # fp16 max (acc init / reduce seed)
QG = 4                         # m-tiles per batched DVE group

_CACHE = {}


def _build():
    nc = bacc.Bacc("TRN2", target_bir_lowering=False, debug=False,
                   num_devices=NCORES)

    tA = nc.dram_tensor("tA", [K11, M], BF16, kind="ExternalInput")
    sA = nc.dram_tensor("sA", [K11, CHUNK], BF16, kind="ExternalInput")
    ntd = nc.dram_tensor("nt", [128, MI], F32, kind="ExternalInput")
    identh = nc.dram_tensor("identh", [128, 128], F16, kind="ExternalInput")
    kp_lhsT = nc.dram_tensor("kp_lhsT", [4, 2 * 3], F32, kind="ExternalInput")
    kp_rhs = nc.dram_tensor("kp_rhs", [4, 2 * KP], F32, kind="ExternalInput")
    tgt_kp = nc.dram_tensor("tgt_kp", [3, 2 * KP], F32, kind="ExternalInput")
    knn_src = nc.dram_tensor("knn_src", [128, 2 * 192], F32, kind="ExternalInput")
    knn_tgt = nc.dram_tensor("knn_tgt", [128, 2 * 192], F32, kind="ExternalInput")

    colmin_o = nc.dram_tensor("colmin", [128, NBLK], F32, kind="ExternalOutput")
    rowmin_o = nc.dram_tensor("rowmin", [128, MI], F32, kind="ExternalOutput")
    misc_o = nc.dram_tensor("misc", [128, 4], F32, kind="ExternalOutput")

    with tile.TileContext(nc) as tc:
        with (
            tc.tile_pool(name="const", bufs=1) as const,
            tc.tile_pool(name="sc", bufs=2) as sc,
        ):
            tA_sb = const.tile([K11, M], BF16)
            sA_sb = const.tile([K11, CHUNK], BF16)
            nt_sb = const.tile([128, MI], F32)
            idh = const.tile([128, 128], F16)
            acc4 = const.tile([128, QG, CHUNK], F16)
            acc = const.tile([128, CHUNK], F16)          # merged accumulator
            f1q = const.tile([128, QG, CHUNK // 2], F16)
            f2q = const.tile([128, QG, CHUNK // 4], F16)
            f3q = const.tile([128, QG, CHUNK // 8], F16)
            f4q = const.tile([128, QG, CHUNK // 16], F16)
            actwarm = const.tile([1, 1], F32)
            rowbuf = const.tile([128, MI], F32)
            colmin_sb = const.tile([128, NBLK], F32)
            misc_sb = const.tile([128, 4], F32)

            # DMA order matters for pipeline fill: the first m-tiles need
            # only sA, the head of tA, and nt — ship those first
            nc.sync.dma_start(out=sA_sb[:], in_=sA[:])
            nc.sync.dma_start(out=tA_sb[:, :1024], in_=tA[:, :1024])
            nc.sync.dma_start(out=nt_sb[:], in_=ntd[:])
            nc.sync.dma_start(out=tA_sb[:, 1024:], in_=tA[:, 1024:])
            nc.sync.dma_start(out=idh[:], in_=identh[:])
            nc.gpsimd.memset(acc4[:], F16BIG)
            nc.gpsimd.memset(misc_sb[:], 0.0)
            # absorb the one-time ACT table load while DMAs are in flight
            # (reads idh, which only needs its DMA — no engine dependency)
            nc.scalar.activation(out=actwarm[:], in_=idh[0:1, 0:1],
                                 func=AF.Identity, bias=0.0, scale=1.0)

            # ---- main loop: Q = -2 t.s + ||s||^2 per 128-row m-tile ----
            with (
                tc.tile_pool(name="psum_main", bufs=2, space="PSUM") as pm,
                tc.tile_pool(name="qpool", bufs=3) as qp,
            ):
                # prologue: first QG tiles unbatched so DVE starts after
                # tile 0 instead of tile QG-1 (cuts the pipeline-fill gap)
                for mi in range(QG):
                    pt = pm.tile([128, CHUNK], F32, tag="pt")
                    for nj in range(NJ):
                        nc.tensor.matmul(
                            pt[:, nj * 512:(nj + 1) * 512],
                            lhsT=tA_sb[:, mi * 128:(mi + 1) * 128],
                            rhs=sA_sb[:, nj * 512:(nj + 1) * 512],
                            start=True, stop=True,
                        )
                    qn1 = qp.tile([128, CHUNK], F16, tag=f"qn1_{mi}")
                    nc.scalar.activation(
                        out=qn1[:], in_=pt[:], func=AF.Identity,
                        bias=nt_sb[:, mi:mi + 1], scale=1.0)
                    # acc4 slot mi starts at BIG, so min(BIG, qn) == copy
                    nc.vector.tensor_copy(out=acc4[:, mi, :], in_=qn1[:])
                    nc.vector.tensor_tensor(
                        out=f1q[:, 0, :], in0=qn1[:, :CHUNK // 2],
                        in1=qn1[:, CHUNK // 2:], op=AL.min)
                    nc.vector.tensor_tensor(
                        out=f2q[:, 0, :], in0=f1q[:, 0, :CHUNK // 4],
                        in1=f1q[:, 0, CHUNK // 4:], op=AL.min)
                    nc.vector.tensor_reduce(
                        out=rowbuf[:, mi:mi + 1], in_=f2q[:, 0, :],
                        axis=mybir.AxisListType.X, op=AL.min)

                for qg in range(1, MI // QG):
                    qn4 = qp.tile([128, QG, CHUNK], F16, tag="qn4")
                    for i in range(QG):
                        mi = qg * QG + i
                        pt = pm.tile([128, CHUNK], F32, tag="pt")
                        for nj in range(NJ):
                            nc.tensor.matmul(
                                pt[:, nj * 512:(nj + 1) * 512],
                                lhsT=tA_sb[:, mi * 128:(mi + 1) * 128],
                                rhs=sA_sb[:, nj * 512:(nj + 1) * 512],
                                start=True, stop=True,
                            )
                        # qn = fp16(Q + ||t||^2) : PSUM -> SBUF on ScalarE
                        nc.scalar.activation(
                            out=qn4[:, i, :], in_=pt[:], func=AF.Identity,
                            bias=nt_sb[:, mi:mi + 1], scale=1.0)
                    # colmin accumulate (DVE, fp16 2x, 4 tiles at once)
                    nc.vector.tensor_tensor(
                        out=acc4[:], in0=acc4[:], in1=qn4[:], op=AL.min)
                    # rowmin: two 2x fold mins, then one 1x reduce -> 4 cols
                    nc.vector.tensor_tensor(
                        out=f1q[:], in0=qn4[:, :, :CHUNK // 2],
                        in1=qn4[:, :, CHUNK // 2:], op=AL.min)
                    nc.vector.tensor_tensor(
                        out=f2q[:], in0=f1q[:, :, :CHUNK // 4],
                        in1=f1q[:, :, CHUNK // 4:], op=AL.min)
                    nc.vector.tensor_tensor(
                        out=f3q[:], in0=f2q[:, :, :CHUNK // 8],
                        in1=f2q[:, :, CHUNK // 8:], op=AL.min)
                    nc.vector.tensor_tensor(
                        out=f4q[:], in0=f3q[:, :, :CHUNK // 16],
                        in1=f3q[:, :, CHUNK // 16:], op=AL.min)
                    nc.vector.tensor_reduce(
                        out=rowbuf[:, qg * QG:(qg + 1) * QG], in_=f4q[:],
                        axis=mybir.AxisListType.X, op=AL.min)
                # merge the 4 group accumulators into one [128, CHUNK]
                nc.vector.tensor_tensor(
                    out=acc4[:, 0, :], in0=acc4[:, 0, :], in1=acc4[:, 1, :],
                    op=AL.min)
                nc.vector.tensor_tensor(
                    out=acc4[:, 2, :], in0=acc4[:, 2, :], in1=acc4[:, 3, :],
                    op=AL.min)
                nc.vector.tensor_tensor(
                    out=acc[:], in0=acc4[:, 0, :], in1=acc4[:, 2, :],
                    op=AL.min)

            with tc.tile_pool(name="psum_fin", bufs=2, space="PSUM") as pf:
                # tiny keypoint / knn losses (both batch elements) — emitted
                # first so DVE has work while PE runs the acc transposes
                kp_l = const.tile([4, 2 * 3], F32)
                kp_r = const.tile([4, 2 * KP], F32)
                kp_t = const.tile([3, 2 * KP], F32)
                ks = const.tile([128, 2 * 192], F32)
                kt = const.tile([128, 2 * 192], F32)
                nc.sync.dma_start(out=kp_l[:], in_=kp_lhsT[:])
                nc.sync.dma_start(out=kp_r[:], in_=kp_rhs[:])
                nc.sync.dma_start(out=kp_t[:], in_=tgt_kp[:])
                nc.sync.dma_start(out=ks[:], in_=knn_src[:])
                nc.sync.dma_start(out=kt[:], in_=knn_tgt[:])
                for b in range(B):
                    pt2 = pf.tile([3, KP], F32, tag="kp")
                    nc.tensor.matmul(
                        pt2[:], lhsT=kp_l[:, b * 3:(b + 1) * 3],
                        rhs=kp_r[:, b * KP:(b + 1) * KP],
                        start=True, stop=True)
                    diff = sc.tile([3, KP], F32, tag="kdiff")
                    nc.vector.tensor_sub(diff[:], pt2[:],
                                         kp_t[:, b * KP:(b + 1) * KP])
                    nc.vector.tensor_mul(diff[:], diff[:], diff[:])
                    nc.vector.tensor_reduce(
                        out=misc_sb[0:3, b:b + 1], in_=diff[:],
                        axis=mybir.AxisListType.X, op=AL.add)
                    diff2 = sc.tile([128, 192], F32, tag="ndiff")
                    nc.vector.tensor_sub(diff2[:], ks[:, b * 192:(b + 1) * 192],
                                         kt[:, b * 192:(b + 1) * 192])
                    nc.vector.tensor_mul(diff2[:], diff2[:], diff2[:])
                    nc.vector.tensor_reduce(
                        out=misc_sb[:, 2 + b:3 + b], in_=diff2[:],
                        axis=mybir.AxisListType.X, op=AL.add)

                # partition-axis min of acc: PE transposes + one wide reduce
                tp = pf.tile([128, NBLK, 128], F16)
                for blk in range(NBLK):
                    nc.tensor.transpose(tp[:, blk, :],
                                        acc[:, blk * 128:(blk + 1) * 128],
                                        idh[:])
                nc.vector.tensor_reduce(
                    out=colmin_sb[:], in_=tp[:],
                    axis=mybir.AxisListType.X, op=AL.min)

            nc.sync.dma_start(out=colmin_o[:], in_=colmin_sb[:])
            nc.sync.dma_start(out=rowmin_o[:], in_=rowbuf[:])
            nc.sync.dma_start(out=misc_o[:], in_=misc_sb[:])

    nc.compile()
    return nc


def _get_nc():
    if "nc" not in _CACHE:
        _CACHE["nc"] = _build()
    return _CACHE["nc"]


def _prepare_in_maps(src_keypoints, tgt_keypoints, rotation_ab, translation_ab,
                     src_keypoints_knn, tgt_keypoints_knn, src_transformed, tgt):
    f = np.float32
    st = np.ascontiguousarray(np.asarray(src_transformed, dtype=f))
    tg = np.ascontiguousarray(np.asarray(tgt, dtype=f))
    skp = np.asarray(src_keypoints, dtype=f)
    tkp = np.asarray(tgt_keypoints, dtype=f)
    rot = np.asarray(rotation_ab, dtype=f)
    tra = np.asarray(translation_ab, dtype=f)
    sknn = np.asarray(src_keypoints_knn, dtype=f)
    tknn = np.asarray(tgt_keypoints_knn, dtype=f)

    identh = np.eye(128, dtype=np.float16)
    kp_lhsT = np.zeros((4, 2 * 3), dtype=f)
    kp_rhs = np.zeros((4, 2 * KP), dtype=f)
    tgt_kp = np.zeros((3, 2 * KP), dtype=f)
    knn_src = np.zeros((128, 2 * 192), dtype=f)
    knn_tgt = np.zeros((128, 2 * 192), dtype=f)
    for b in range(B):
        kp_lhsT[0:3, b * 3:(b + 1) * 3] = rot[b].T
        kp_lhsT[3, b * 3:(b + 1) * 3] = tra[b]
        kp_rhs[0:3, b * KP:(b + 1) * KP] = skp[b]
        kp_rhs[3, b * KP:(b + 1) * KP] = 1.0
        tgt_kp[:, b * KP:(b + 1) * KP] = tkp[b]
        knn_src[:, b * 192:(b + 1) * 192] = sknn[b].reshape(128, 192)
        knn_tgt[:, b * 192:(b + 1) * 192] = tknn[b].reshape(128, 192)

    # t-side (tgt, M axis): 2-way bf16 split of -2*t, plus ||t||^2 in fp32
    tAs, nts = [], []
    for b in range(B):
        t2 = (-2.0 * tg[b]).astype(f)                       # [3, M]
        th = t2.astype(BF16NP)
        tl = (t2 - th.astype(f)).astype(BF16NP)
        tA = np.empty((K11, M), dtype=BF16NP)
        tA[0:3] = th
        tA[3:6] = th
        tA[6:9] = tl
        tA[9:11] = BF16NP(1.0)
        tAs.append(tA)
        nt = (tg[b].astype(np.float64) ** 2).sum(axis=0).astype(f)   # [M]
        nts.append(np.ascontiguousarray(nt.reshape(MI, 128).T))      # [128, MI]

    shared = {
        "identh": identh, "kp_lhsT": kp_lhsT, "kp_rhs": kp_rhs,
        "tgt_kp": tgt_kp, "knn_src": knn_src, "knn_tgt": knn_tgt,
    }
    in_maps = []
    for ci in range(NCORES):
        b, j = divmod(ci, NSHARDS)
        s = st[b][:, j * CHUNK:(j + 1) * CHUNK]             # [3, CHUNK]
        sh = s.astype(BF16NP)
        sl = (s - sh.astype(f)).astype(BF16NP)
        ns64 = (s.astype(np.float64) ** 2).sum(axis=0)      # [CHUNK]
        nsh = ns64.astype(BF16NP)
        nsl = (ns64 - nsh.astype(np.float64)).astype(BF16NP)
        sA = np.empty((K11, CHUNK), dtype=BF16NP)
        sA[0:3] = sh
        sA[3:6] = sl
        sA[6:9] = sh
        sA[9] = nsh
        sA[10] = nsl
        m = dict(shared)
        m["tA"] = tAs[b]
        m["sA"] = sA
        m["nt"] = nts[b]
        in_maps.append(m)
    return in_maps


def _huber(x, c):
    return np.where(x < c, 0.5 * x * x, c * x - 0.5 * c * c)


def _postprocess(results):
    c = np.float64(MARGIN)
    loss1 = np.float64(0.0)
    loss2 = np.float64(0.0)
    for b in range(B):
        rowmins = []
        for j in range(NSHARDS):
            r = results[b * NSHARDS + j]
            colmin = np.asarray(r["colmin"], dtype=np.float64).T.ravel()
            loss1 += _huber(colmin, c).sum()
            rowmins.append(np.asarray(r["rowmin"], dtype=np.float64).T.ravel())
        rm = np.minimum.reduce(rowmins)
        loss2 += _huber(rm, c).sum()
    gal = loss1 + loss2

    misc = np.asarray(results[0]["misc"], dtype=np.float64)
    kp_loss = (misc[0:3, 0].sum() + misc[0:3, 1].sum()) / B
    knn_loss = (misc[:, 2].sum() + misc[:, 3].sum()) / (B * KNN)
    ncl = knn_loss + kp_loss
    return np.float32(ncl), np.float32(gal)


def run_device(in_maps, **kw):
    nc = _get_nc()
    return run_bass_kernel_spmd(nc, in_maps, list(range(NCORES)), **kw)


def kernel(src_keypoints, tgt_keypoints, rotation_ab, translation_ab,
           src_keypoints_knn, tgt_keypoints_knn, k, src_transformed, tgt,
           **_unused):
    in_maps = _prepare_in_maps(src_keypoints, tgt_keypoints, rotation_ab,
                               translation_ab, src_keypoints_knn,
                               tgt_keypoints_knn, src_transformed, tgt)
    res = run_device(in_maps)
    return _postprocess(res.results)


# revision 29
# speedup vs baseline: 1.0262x; 1.0188x over previous
"""RIENet loss kernel (keypoint/KNN MSE + global-align Huber-min loss) on 8 trn2 cores.

Sharding: core ci -> (b = ci // 4, n-chunk j = ci % 4).  Each core holds the full
tgt[b] (M=8192 points) and a 2048-column chunk of src_transformed[b] (N axis).
  loss_1 (min over M per src point): complete locally per core.
  loss_2 (min over N per tgt point): per-core partial min over its chunk;
          host min-reduces the 4 chunks per batch element.

Device kernel per core (v3 — host-prepped operands, fp16 min path):
  Operand prep moved to the host: the 2-way bf16 splits of (-2*t) and s, the
  ||s||^2 split rows, and ||t||^2 are computed in numpy and DMA'd in directly
  (kills the on-device transpose/DMA preamble of v2).
  Q[m, n] = -2 t_m . s_n + ||s_n||^2 from one K=11 bf16 matmul per (m-tile,
  512-col block); the dropped tl*sl term is ~2^-17 relative, far inside the
  2e-2 tolerance.
  Per 128-row m-tile: ScalarE writes qn = fp16(Q + ||t_m||^2) (ACTIVATE
  Identity with a per-partition bias AP) into one slot of a 4-tile group
  buffer.  DVE work is batched per group of 4 m-tiles to amortize the
  ~150-cycle per-op overhead and the 1x-only tensor_reduce:
    acc4 = min(acc4, qn4)                       [128,4,2048] fp16 2x
    f1q  = min(qn4[..lo], qn4[..hi])            [128,4,1024] 2x
    f2q  = min(f1q[..lo], f1q[..hi])            [128,4,512]  2x
    rowbuf[:, 4q:4q+4] = reduce_min_X(f2q)      1x
  (tensor_tensor_reduce would fuse fold+reduce but its min/min form
  crashes the exec unit on hw; gpsimd tensor_tensor is rejected by
  walrus codegen, so no Pool-engine offload either)
  fp16 is safe: mins are order-statistics (abs err ~ val*2^-12, and loss
  sensitivity d huber/dx <= max(x, 0.1)).
  Finalize: PE-transpose acc into PSUM, one 2048-wide DVE min-reduce ->
  per-n colmin.  Tiny keypoint/KNN MSE losses run on-device on every core.
"""

import os
import numpy as np


def _ensure_path():
    try:
        import concourse  # noqa: F401
    except ImportError:
        import sys
        for p in ("/opt/trn_rl_repo", "/root/.axon_site/_ro/trn_rl_repo"):
            if os.path.isdir(p) and p not in sys.path:
                sys.path.insert(0, p)


_ensure_path()

import concourse.bass as bass  # noqa: E402
import concourse.bacc as bacc  # noqa: E402
import concourse.tile as tile  # noqa: E402
import concourse.mybir as mybir  # noqa: E402
from concourse.bass_utils import run_bass_kernel_spmd  # noqa: E402

import ml_dtypes  # noqa: E402

F32 = mybir.dt.float32
F16 = mybir.dt.float16
BF16 = mybir.dt.bfloat16
AL = mybir.AluOpType
AF = mybir.ActivationFunctionType

BF16NP = ml_dtypes.bfloat16

MARGIN = 0.1
B, KP, KNN, N, M = 2, 256, 32, 8192, 8192
NCORES = 8
NSHARDS = NCORES // B          # 4 n-chunks per batch element
CHUNK = N // NSHARDS           # 2048
NJ = CHUNK // 512              # 4 psum banks per m-tile
MI = M // 128                  # 64 m-tiles
NBLK = CHUNK // 128            # 16 column blocks for the final transpose
K11 = 11                       # 9 bf16-split product rows + 2 ||s||^2 rows
F16BIG = 65504.0               # fp16 max (acc init / reduce seed)
QG = 4                         # m-tiles per batched DVE group

_CACHE = {}


def _build():
    nc = bacc.Bacc("TRN2", target_bir_lowering=False, debug=False,
                   num_devices=NCORES)

    tA = nc.dram_tensor("tA", [K11, M], BF16, kind="ExternalInput")
    sA = nc.dram_tensor("sA", [K11, CHUNK], BF16, kind="ExternalInput")
    ntd = nc.dram_tensor("nt", [128, MI], F32, kind="ExternalInput")
    identh = nc.dram_tensor("identh", [128, 128], F16, kind="ExternalInput")
    kp_lhsT = nc.dram_tensor("kp_lhsT", [4, 2 * 3], F32, kind="ExternalInput")
    kp_rhs = nc.dram_tensor("kp_rhs", [4, 2 * KP], F32, kind="ExternalInput")
    tgt_kp = nc.dram_tensor("tgt_kp", [3, 2 * KP], F32, kind="ExternalInput")
    knn_src = nc.dram_tensor("knn_src", [128, 2 * 192], F32, kind="ExternalInput")
    knn_tgt = nc.dram_tensor("knn_tgt", [128, 2 * 192], F32, kind="ExternalInput")

    # raw colmin partials [p, slot, n]; host takes the (p, slot) min
    colmin_o = nc.dram_tensor("colmin", [128, QG * CHUNK], F16,
                              kind="ExternalOutput")
    rowmin_o = nc.dram_tensor("rowmin", [128, MI], F32, kind="ExternalOutput")
    misc_o = nc.dram_tensor("misc", [128, 4], F32, kind="ExternalOutput")

    with tile.TileContext(nc) as tc:
        with (
            tc.tile_pool(name="const", bufs=1) as const,
            tc.tile_pool(name="sc", bufs=2) as sc,
        ):
            tA_sb = const.tile([K11, M], BF16)
            sA_sb = const.tile([K11, CHUNK], BF16)
            nt_sb = const.tile([128, MI], F32)
            idh = const.tile([128, 128], F16)
            acc4 = const.tile([128, QG, CHUNK], F16)
            f1q = const.tile([128, QG, CHUNK // 2], F16)
            f2q = const.tile([128, QG, CHUNK // 4], F16)
            f3q = const.tile([128, QG, CHUNK // 8], F16)
            f4q = const.tile([128, QG, CHUNK // 16], F16)
            actwarm = const.tile([1, 1], F32)
            rowbuf = const.tile([128, MI], F32)
            misc_sb = const.tile([128, 4], F32)

            # DMA order matters for pipeline fill: the first m-tiles need
            # only sA, the head of tA, and nt — ship those first
            nc.sync.dma_start(out=sA_sb[:], in_=sA[:])
            nc.sync.dma_start(out=tA_sb[:, :1024], in_=tA[:, :1024])
            nc.sync.dma_start(out=nt_sb[:], in_=ntd[:])
            nc.sync.dma_start(out=tA_sb[:, 1024:], in_=tA[:, 1024:])
            nc.sync.dma_start(out=idh[:], in_=identh[:])
            nc.gpsimd.memset(acc4[:], F16BIG)
            nc.gpsimd.memset(misc_sb[:], 0.0)
            # absorb the one-time ACT table load while DMAs are in flight
            # (reads idh, which only needs its DMA — no engine dependency)
            nc.scalar.activation(out=actwarm[:], in_=idh[0:1, 0:1],
                                 func=AF.Identity, bias=0.0, scale=1.0)

            # ---- main loop: Q = -2 t.s + ||s||^2 per 128-row m-tile ----
            with (
                tc.tile_pool(name="psum_main", bufs=2, space="PSUM") as pm,
                tc.tile_pool(name="qpool", bufs=3) as qp,
            ):
                # prologue: first QG tiles unbatched so DVE starts after
                # tile 0 instead of tile QG-1 (cuts the pipeline-fill gap)
                for mi in range(QG):
                    pt = pm.tile([128, CHUNK], F32, tag="pt")
                    for nj in range(NJ):
                        nc.tensor.matmul(
                            pt[:, nj * 512:(nj + 1) * 512],
                            lhsT=tA_sb[:, mi * 128:(mi + 1) * 128],
                            rhs=sA_sb[:, nj * 512:(nj + 1) * 512],
                            start=True, stop=True,
                        )
                    qn1 = qp.tile([128, CHUNK], F16, tag=f"qn1_{mi}")
                    nc.scalar.activation(
                        out=qn1[:], in_=pt[:], func=AF.Identity,
                        bias=nt_sb[:, mi:mi + 1], scale=1.0)
                    # acc4 slot mi starts at BIG, so min(BIG, qn) == copy
                    nc.vector.tensor_copy(out=acc4[:, mi, :], in_=qn1[:])
                    nc.vector.tensor_tensor(
                        out=f1q[:, 0, :], in0=qn1[:, :CHUNK // 2],
                        in1=qn1[:, CHUNK // 2:], op=AL.min)
                    nc.vector.tensor_tensor(
                        out=f2q[:, 0, :], in0=f1q[:, 0, :CHUNK // 4],
                        in1=f1q[:, 0, CHUNK // 4:], op=AL.min)
                    nc.vector.tensor_reduce(
                        out=rowbuf[:, mi:mi + 1], in_=f2q[:, 0, :],
                        axis=mybir.AxisListType.X, op=AL.min)

                for qg in range(1, MI // QG):
                    qn4 = qp.tile([128, QG, CHUNK], F16, tag="qn4")
                    for i in range(QG):
                        mi = qg * QG + i
                        pt = pm.tile([128, CHUNK], F32, tag="pt")
                        for nj in range(NJ):
                            nc.tensor.matmul(
                                pt[:, nj * 512:(nj + 1) * 512],
                                lhsT=tA_sb[:, mi * 128:(mi + 1) * 128],
                                rhs=sA_sb[:, nj * 512:(nj + 1) * 512],
                                start=True, stop=True,
                            )
                        # qn = fp16(Q + ||t||^2) : PSUM -> SBUF on ScalarE
                        nc.scalar.activation(
                            out=qn4[:, i, :], in_=pt[:], func=AF.Identity,
                            bias=nt_sb[:, mi:mi + 1], scale=1.0)
                    # colmin accumulate (DVE, fp16 2x, 4 tiles at once)
                    nc.vector.tensor_tensor(
                        out=acc4[:], in0=acc4[:], in1=qn4[:], op=AL.min)
                    # rowmin: two 2x fold mins, then one 1x reduce -> 4 cols
                    nc.vector.tensor_tensor(
                        out=f1q[:], in0=qn4[:, :, :CHUNK // 2],
                        in1=qn4[:, :, CHUNK // 2:], op=AL.min)
                    nc.vector.tensor_tensor(
                        out=f2q[:], in0=f1q[:, :, :CHUNK // 4],
                        in1=f1q[:, :, CHUNK // 4:], op=AL.min)
                    nc.vector.tensor_tensor(
                        out=f3q[:], in0=f2q[:, :, :CHUNK // 8],
                        in1=f2q[:, :, CHUNK // 8:], op=AL.min)
                    nc.vector.tensor_tensor(
                        out=f4q[:], in0=f3q[:, :, :CHUNK // 16],
                        in1=f3q[:, :, CHUNK // 16:], op=AL.min)
                    nc.vector.tensor_reduce(
                        out=rowbuf[:, qg * QG:(qg + 1) * QG], in_=f4q[:],
                        axis=mybir.AxisListType.X, op=AL.min)
                # colmin partials go to the host raw — the final min over
                # (partition, slot) is cheaper there than merge + PE
                # transposes + another DVE reduce on device
                nc.sync.dma_start(
                    out=colmin_o.rearrange("p (q c) -> p q c", q=QG),
                    in_=acc4[:])

            with tc.tile_pool(name="psum_fin", bufs=2, space="PSUM") as pf:
                # tiny keypoint / knn losses (both batch elements) — emitted
                # first so DVE has work while PE runs the acc transposes
                kp_l = const.tile([4, 2 * 3], F32)
                kp_r = const.tile([4, 2 * KP], F32)
                kp_t = const.tile([3, 2 * KP], F32)
                ks = const.tile([128, 2 * 192], F32)
                kt = const.tile([128, 2 * 192], F32)
                nc.sync.dma_start(out=kp_l[:], in_=kp_lhsT[:])
                nc.sync.dma_start(out=kp_r[:], in_=kp_rhs[:])
                nc.sync.dma_start(out=kp_t[:], in_=tgt_kp[:])
                nc.sync.dma_start(out=ks[:], in_=knn_src[:])
                nc.sync.dma_start(out=kt[:], in_=knn_tgt[:])
                for b in range(B):
                    pt2 = pf.tile([3, KP], F32, tag="kp")
                    nc.tensor.matmul(
                        pt2[:], lhsT=kp_l[:, b * 3:(b + 1) * 3],
                        rhs=kp_r[:, b * KP:(b + 1) * KP],
                        start=True, stop=True)
                    diff = sc.tile([3, KP], F32, tag="kdiff")
                    nc.vector.tensor_sub(diff[:], pt2[:],
                                         kp_t[:, b * KP:(b + 1) * KP])
                    nc.vector.tensor_mul(diff[:], diff[:], diff[:])
                    nc.vector.tensor_reduce(
                        out=misc_sb[0:3, b:b + 1], in_=diff[:],
                        axis=mybir.AxisListType.X, op=AL.add)
                    diff2 = sc.tile([128, 192], F32, tag="ndiff")
                    nc.vector.tensor_sub(diff2[:], ks[:, b * 192:(b + 1) * 192],
                                         kt[:, b * 192:(b + 1) * 192])
                    nc.vector.tensor_mul(diff2[:], diff2[:], diff2[:])
                    nc.vector.tensor_reduce(
                        out=misc_sb[:, 2 + b:3 + b], in_=diff2[:],
                        axis=mybir.AxisListType.X, op=AL.add)

            nc.sync.dma_start(out=rowmin_o[:], in_=rowbuf[:])
            nc.sync.dma_start(out=misc_o[:], in_=misc_sb[:])

    nc.compile()
    return nc


def _get_nc():
    if "nc" not in _CACHE:
        _CACHE["nc"] = _build()
    return _CACHE["nc"]


def _prepare_in_maps(src_keypoints, tgt_keypoints, rotation_ab, translation_ab,
                     src_keypoints_knn, tgt_keypoints_knn, src_transformed, tgt):
    f = np.float32
    st = np.ascontiguousarray(np.asarray(src_transformed, dtype=f))
    tg = np.ascontiguousarray(np.asarray(tgt, dtype=f))
    skp = np.asarray(src_keypoints, dtype=f)
    tkp = np.asarray(tgt_keypoints, dtype=f)
    rot = np.asarray(rotation_ab, dtype=f)
    tra = np.asarray(translation_ab, dtype=f)
    sknn = np.asarray(src_keypoints_knn, dtype=f)
    tknn = np.asarray(tgt_keypoints_knn, dtype=f)

    identh = np.eye(128, dtype=np.float16)
    kp_lhsT = np.zeros((4, 2 * 3), dtype=f)
    kp_rhs = np.zeros((4, 2 * KP), dtype=f)
    tgt_kp = np.zeros((3, 2 * KP), dtype=f)
    knn_src = np.zeros((128, 2 * 192), dtype=f)
    knn_tgt = np.zeros((128, 2 * 192), dtype=f)
    for b in range(B):
        kp_lhsT[0:3, b * 3:(b + 1) * 3] = rot[b].T
        kp_lhsT[3, b * 3:(b + 1) * 3] = tra[b]
        kp_rhs[0:3, b * KP:(b + 1) * KP] = skp[b]
        kp_rhs[3, b * KP:(b + 1) * KP] = 1.0
        tgt_kp[:, b * KP:(b + 1) * KP] = tkp[b]
        knn_src[:, b * 192:(b + 1) * 192] = sknn[b].reshape(128, 192)
        knn_tgt[:, b * 192:(b + 1) * 192] = tknn[b].reshape(128, 192)

    # t-side (tgt, M axis): 2-way bf16 split of -2*t, plus ||t||^2 in fp32
    tAs, nts = [], []
    for b in range(B):
        t2 = (-2.0 * tg[b]).astype(f)                       # [3, M]
        th = t2.astype(BF16NP)
        tl = (t2 - th.astype(f)).astype(BF16NP)
        tA = np.empty((K11, M), dtype=BF16NP)
        tA[0:3] = th
        tA[3:6] = th
        tA[6:9] = tl
        tA[9:11] = BF16NP(1.0)
        tAs.append(tA)
        nt = (tg[b].astype(np.float64) ** 2).sum(axis=0).astype(f)   # [M]
        nts.append(np.ascontiguousarray(nt.reshape(MI, 128).T))      # [128, MI]

    shared = {
        "identh": identh, "kp_lhsT": kp_lhsT, "kp_rhs": kp_rhs,
        "tgt_kp": tgt_kp, "knn_src": knn_src, "knn_tgt": knn_tgt,
    }
    in_maps = []
    for ci in range(NCORES):
        b, j = divmod(ci, NSHARDS)
        s = st[b][:, j * CHUNK:(j + 1) * CHUNK]             # [3, CHUNK]
        sh = s.astype(BF16NP)
        sl = (s - sh.astype(f)).astype(BF16NP)
        ns64 = (s.astype(np.float64) ** 2).sum(axis=0)      # [CHUNK]
        nsh = ns64.astype(BF16NP)
        nsl = (ns64 - nsh.astype(np.float64)).astype(BF16NP)
        sA = np.empty((K11, CHUNK), dtype=BF16NP)
        sA[0:3] = sh
        sA[3:6] = sl
        sA[6:9] = sh
        sA[9] = nsh
        sA[10] = nsl
        m = dict(shared)
        m["tA"] = tAs[b]
        m["sA"] = sA
        m["nt"] = nts[b]
        in_maps.append(m)
    return in_maps


def _huber(x, c):
    return np.where(x < c, 0.5 * x * x, c * x - 0.5 * c * c)


def _postprocess(results):
    c = np.float64(MARGIN)
    loss1 = np.float64(0.0)
    loss2 = np.float64(0.0)
    for b in range(B):
        rowmins = []
        for j in range(NSHARDS):
            r = results[b * NSHARDS + j]
            # raw partials [p, slot*CHUNK]: final min over (p, slot) here
            colmin = (np.asarray(r["colmin"], dtype=np.float64)
                      .reshape(128, QG, CHUNK).min(axis=(0, 1)))
            loss1 += _huber(colmin, c).sum()
            rowmins.append(np.asarray(r["rowmin"], dtype=np.float64).T.ravel())
        rm = np.minimum.reduce(rowmins)
        loss2 += _huber(rm, c).sum()
    gal = loss1 + loss2

    misc = np.asarray(results[0]["misc"], dtype=np.float64)
    kp_loss = (misc[0:3, 0].sum() + misc[0:3, 1].sum()) / B
    knn_loss = (misc[:, 2].sum() + misc[:, 3].sum()) / (B * KNN)
    ncl = knn_loss + kp_loss
    return np.float32(ncl), np.float32(gal)


def run_device(in_maps, **kw):
    nc = _get_nc()
    return run_bass_kernel_spmd(nc, in_maps, list(range(NCORES)), **kw)


def kernel(src_keypoints, tgt_keypoints, rotation_ab, translation_ab,
           src_keypoints_knn, tgt_keypoints_knn, k, src_transformed, tgt,
           **_unused):
    in_maps = _prepare_in_maps(src_keypoints, tgt_keypoints, rotation_ab,
                               translation_ab, src_keypoints_knn,
                               tgt_keypoints_knn, src_transformed, tgt)
    res = run_device(in_maps)
    return _postprocess(res.results)


# revision 35
# speedup vs baseline: 1.0294x; 1.0031x over previous
"""RIENet loss kernel (keypoint/KNN MSE + global-align Huber-min loss) on 8 trn2 cores.

Sharding: core ci -> (b = ci // 4, n-chunk j = ci % 4).  Each core holds the full
tgt[b] (M=8192 points) and a 2048-column chunk of src_transformed[b] (N axis).
  loss_1 (min over M per src point): complete locally per core.
  loss_2 (min over N per tgt point): per-core partial min over its chunk;
          host min-reduces the 4 chunks per batch element.

Device kernel per core (v3 — host-prepped operands, fp16 min path):
  Operand prep moved to the host: the 2-way bf16 splits of (-2*t) and s, the
  ||s||^2 split rows, and ||t||^2 are computed in numpy and DMA'd in directly
  (kills the on-device transpose/DMA preamble of v2).
  Q[m, n] = -2 t_m . s_n + ||s_n||^2 from one K=11 bf16 matmul per (m-tile,
  512-col block); the dropped tl*sl term is ~2^-17 relative, far inside the
  2e-2 tolerance.
  Per 128-row m-tile: ScalarE writes qn = fp16(Q + ||t_m||^2) (ACTIVATE
  Identity with a per-partition bias AP) into one slot of a 4-tile group
  buffer.  DVE work is batched per group of 4 m-tiles to amortize the
  ~150-cycle per-op overhead and the 1x-only tensor_reduce:
    acc4 = min(acc4, qn4)                       [128,4,2048] fp16 2x
    f1q  = min(qn4[..lo], qn4[..hi])            [128,4,1024] 2x
    f2q  = min(f1q[..lo], f1q[..hi])            [128,4,512]  2x
    rowbuf[:, 4q:4q+4] = reduce_min_X(f2q)      1x
  (tensor_tensor_reduce would fuse fold+reduce but its min/min form
  crashes the exec unit on hw; gpsimd tensor_tensor is rejected by
  walrus codegen, so no Pool-engine offload either)
  fp16 is safe: mins are order-statistics (abs err ~ val*2^-12, and loss
  sensitivity d huber/dx <= max(x, 0.1)).
  Finalize: PE-transpose acc into PSUM, one 2048-wide DVE min-reduce ->
  per-n colmin.  Tiny keypoint/KNN MSE losses run on-device on every core.
"""

import os
import numpy as np


def _ensure_path():
    try:
        import concourse  # noqa: F401
    except ImportError:
        import sys
        for p in ("/opt/trn_rl_repo", "/root/.axon_site/_ro/trn_rl_repo"):
            if os.path.isdir(p) and p not in sys.path:
                sys.path.insert(0, p)


_ensure_path()

import concourse.bass as bass  # noqa: E402
import concourse.bacc as bacc  # noqa: E402
import concourse.tile as tile  # noqa: E402
import concourse.mybir as mybir  # noqa: E402
from concourse.bass_utils import run_bass_kernel_spmd  # noqa: E402

import ml_dtypes  # noqa: E402

F32 = mybir.dt.float32
F16 = mybir.dt.float16
BF16 = mybir.dt.bfloat16
AL = mybir.AluOpType
AF = mybir.ActivationFunctionType

BF16NP = ml_dtypes.bfloat16

MARGIN = 0.1
B, KP, KNN, N, M = 2, 256, 32, 8192, 8192
NCORES = 8
NSHARDS = NCORES // B          # 4 n-chunks per batch element
CHUNK = N // NSHARDS           # 2048
NJ = CHUNK // 512              # 4 psum banks per m-tile
MI = M // 128                  # 64 m-tiles
NBLK = CHUNK // 128            # 16 column blocks for the final transpose
K11 = 11                       # 9 bf16-split product rows + 2 ||s||^2 rows
F16BIG = 65504.0               # fp16 max (acc init / reduce seed)
QG = 4                         # m-tiles per batched DVE group
PROLOG = QG                    # unbatched warm-up tiles (must equal QG)

_CACHE = {}


def _build():
    nc = bacc.Bacc("TRN2", target_bir_lowering=False, debug=False,
                   num_devices=NCORES)

    tA = nc.dram_tensor("tA", [K11, M], BF16, kind="ExternalInput")
    sA = nc.dram_tensor("sA", [K11, CHUNK], BF16, kind="ExternalInput")
    ntd = nc.dram_tensor("nt", [128, MI], F32, kind="ExternalInput")
    identh = nc.dram_tensor("identh", [128, 128], F16, kind="ExternalInput")
    kp_lhsT = nc.dram_tensor("kp_lhsT", [4, 2 * 3], F32, kind="ExternalInput")
    kp_rhs = nc.dram_tensor("kp_rhs", [4, 2 * KP], F32, kind="ExternalInput")
    tgt_kp = nc.dram_tensor("tgt_kp", [3, 2 * KP], F32, kind="ExternalInput")
    knn_src = nc.dram_tensor("knn_src", [128, 2 * 192], F32, kind="ExternalInput")
    knn_tgt = nc.dram_tensor("knn_tgt", [128, 2 * 192], F32, kind="ExternalInput")

    # raw colmin partials [p, slot, n]; host takes the (p, slot) min
    colmin_o = nc.dram_tensor("colmin", [128, QG * CHUNK], F16,
                              kind="ExternalOutput")
    rowmin_o = nc.dram_tensor("rowmin", [128, MI], F32, kind="ExternalOutput")
    misc_o = nc.dram_tensor("misc", [128, 4], F32, kind="ExternalOutput")

    with tile.TileContext(nc) as tc:
        with (
            tc.tile_pool(name="const", bufs=1) as const,
            tc.tile_pool(name="sc", bufs=2) as sc,
        ):
            tA_sb = const.tile([K11, M], BF16)
            sA_sb = const.tile([K11, CHUNK], BF16)
            nt_sb = const.tile([128, MI], F32)
            idh = const.tile([128, 128], F16)
            acc4 = const.tile([128, QG, CHUNK], F16)
            f1q = const.tile([128, QG, CHUNK // 2], F16)
            f2q = const.tile([128, QG, CHUNK // 4], F16)
            f3q = const.tile([128, QG, CHUNK // 8], F16)
            f4q = const.tile([128, QG, CHUNK // 16], F16)
            actwarm = const.tile([1, 1], F32)
            rowbuf = const.tile([128, MI], F32)
            misc_sb = const.tile([128, 4], F32)

            # DMA order matters for pipeline fill: the first m-tiles need
            # only sA, the head of tA, and nt — ship those first
            nc.sync.dma_start(out=sA_sb[:], in_=sA[:])
            nc.sync.dma_start(out=tA_sb[:, :1024], in_=tA[:, :1024])
            nc.sync.dma_start(out=nt_sb[:], in_=ntd[:])
            nc.sync.dma_start(out=tA_sb[:, 1024:], in_=tA[:, 1024:])
            nc.sync.dma_start(out=idh[:], in_=identh[:])
            nc.gpsimd.memset(acc4[:], F16BIG)
            nc.gpsimd.memset(misc_sb[:], 0.0)
            # absorb the one-time ACT table load while DMAs are in flight
            # (reads idh, which only needs its DMA — no engine dependency)
            nc.scalar.activation(out=actwarm[:], in_=idh[0:1, 0:1],
                                 func=AF.Identity, bias=0.0, scale=1.0)

            # ---- main loop: Q = -2 t.s + ||s||^2 per 128-row m-tile ----
            with (
                tc.tile_pool(name="psum_main", bufs=2, space="PSUM") as pm,
                tc.tile_pool(name="qpool", bufs=3) as qp,
            ):
                # prologue: first PROLOG tiles unbatched so DVE starts after
                # tile 0 and stays fed through the ScalarE ramp
                for mi in range(PROLOG):
                    pt = pm.tile([128, CHUNK], F32, tag="pt")
                    for nj in range(NJ):
                        nc.tensor.matmul(
                            pt[:, nj * 512:(nj + 1) * 512],
                            lhsT=tA_sb[:, mi * 128:(mi + 1) * 128],
                            rhs=sA_sb[:, nj * 512:(nj + 1) * 512],
                            start=True, stop=True,
                        )
                    qn1 = qp.tile([128, CHUNK], F16, tag=f"qn1_{mi}")
                    nc.scalar.activation(
                        out=qn1[:], in_=pt[:], func=AF.Identity,
                        bias=nt_sb[:, mi:mi + 1], scale=1.0)
                    # acc4 slot mi starts at BIG, so min(BIG, qn) == copy
                    nc.vector.tensor_copy(out=acc4[:, mi, :], in_=qn1[:])
                    nc.vector.tensor_tensor(
                        out=f1q[:, 0, :], in0=qn1[:, :CHUNK // 2],
                        in1=qn1[:, CHUNK // 2:], op=AL.min)
                    nc.vector.tensor_tensor(
                        out=f2q[:, 0, :], in0=f1q[:, 0, :CHUNK // 4],
                        in1=f1q[:, 0, CHUNK // 4:], op=AL.min)
                    nc.vector.tensor_reduce(
                        out=rowbuf[:, mi:mi + 1], in_=f2q[:, 0, :],
                        axis=mybir.AxisListType.X, op=AL.min)

                for qg in range(1, MI // QG):
                    qn4 = qp.tile([128, QG, CHUNK], F16, tag="qn4")
                    for i in range(QG):
                        mi = qg * QG + i
                        pt = pm.tile([128, CHUNK], F32, tag="pt")
                        for nj in range(NJ):
                            nc.tensor.matmul(
                                pt[:, nj * 512:(nj + 1) * 512],
                                lhsT=tA_sb[:, mi * 128:(mi + 1) * 128],
                                rhs=sA_sb[:, nj * 512:(nj + 1) * 512],
                                start=True, stop=True,
                            )
                        # qn = fp16(Q + ||t||^2) : PSUM -> SBUF on ScalarE
                        nc.scalar.activation(
                            out=qn4[:, i, :], in_=pt[:], func=AF.Identity,
                            bias=nt_sb[:, mi:mi + 1], scale=1.0)
                    # colmin accumulate (DVE, fp16 2x, 4 tiles at once)
                    nc.vector.tensor_tensor(
                        out=acc4[:], in0=acc4[:], in1=qn4[:], op=AL.min)
                    # rowmin: two 2x fold mins, then one 1x reduce -> 4 cols
                    nc.vector.tensor_tensor(
                        out=f1q[:], in0=qn4[:, :, :CHUNK // 2],
                        in1=qn4[:, :, CHUNK // 2:], op=AL.min)
                    nc.vector.tensor_tensor(
                        out=f2q[:], in0=f1q[:, :, :CHUNK // 4],
                        in1=f1q[:, :, CHUNK // 4:], op=AL.min)
                    nc.vector.tensor_tensor(
                        out=f3q[:], in0=f2q[:, :, :CHUNK // 8],
                        in1=f2q[:, :, CHUNK // 8:], op=AL.min)
                    nc.vector.tensor_tensor(
                        out=f4q[:], in0=f3q[:, :, :CHUNK // 16],
                        in1=f3q[:, :, CHUNK // 16:], op=AL.min)
                    nc.vector.tensor_reduce(
                        out=rowbuf[:, qg * QG:(qg + 1) * QG], in_=f4q[:],
                        axis=mybir.AxisListType.X, op=AL.min)
                # colmin partials go to the host raw — the final min over
                # (partition, slot) is cheaper there than merge + PE
                # transposes + another DVE reduce on device
                nc.sync.dma_start(
                    out=colmin_o.rearrange("p (q c) -> p q c", q=QG),
                    in_=acc4[:])

            with tc.tile_pool(name="psum_fin", bufs=2, space="PSUM") as pf:
                # tiny keypoint / knn losses (both batch elements) — emitted
                # first so DVE has work while PE runs the acc transposes
                kp_l = const.tile([4, 2 * 3], F32)
                kp_r = const.tile([4, 2 * KP], F32)
                kp_t = const.tile([3, 2 * KP], F32)
                ks = const.tile([128, 2 * 192], F32)
                kt = const.tile([128, 2 * 192], F32)
                nc.sync.dma_start(out=kp_l[:], in_=kp_lhsT[:])
                nc.sync.dma_start(out=kp_r[:], in_=kp_rhs[:])
                nc.sync.dma_start(out=kp_t[:], in_=tgt_kp[:])
                nc.sync.dma_start(out=ks[:], in_=knn_src[:])
                nc.sync.dma_start(out=kt[:], in_=knn_tgt[:])
                for b in range(B):
                    pt2 = pf.tile([3, KP], F32, tag="kp")
                    nc.tensor.matmul(
                        pt2[:], lhsT=kp_l[:, b * 3:(b + 1) * 3],
                        rhs=kp_r[:, b * KP:(b + 1) * KP],
                        start=True, stop=True)
                    diff = sc.tile([3, KP], F32, tag="kdiff")
                    nc.vector.tensor_sub(diff[:], pt2[:],
                                         kp_t[:, b * KP:(b + 1) * KP])
                    nc.vector.tensor_mul(diff[:], diff[:], diff[:])
                    nc.vector.tensor_reduce(
                        out=misc_sb[0:3, b:b + 1], in_=diff[:],
                        axis=mybir.AxisListType.X, op=AL.add)
                    diff2 = sc.tile([128, 192], F32, tag="ndiff")
                    nc.vector.tensor_sub(diff2[:], ks[:, b * 192:(b + 1) * 192],
                                         kt[:, b * 192:(b + 1) * 192])
                    nc.vector.tensor_mul(diff2[:], diff2[:], diff2[:])
                    nc.vector.tensor_reduce(
                        out=misc_sb[:, 2 + b:3 + b], in_=diff2[:],
                        axis=mybir.AxisListType.X, op=AL.add)

            nc.sync.dma_start(out=rowmin_o[:], in_=rowbuf[:])
            nc.sync.dma_start(out=misc_o[:], in_=misc_sb[:])

    nc.compile()
    return nc


def _get_nc():
    if "nc" not in _CACHE:
        _CACHE["nc"] = _build()
    return _CACHE["nc"]


def _prepare_in_maps(src_keypoints, tgt_keypoints, rotation_ab, translation_ab,
                     src_keypoints_knn, tgt_keypoints_knn, src_transformed, tgt):
    f = np.float32
    st = np.ascontiguousarray(np.asarray(src_transformed, dtype=f))
    tg = np.ascontiguousarray(np.asarray(tgt, dtype=f))
    skp = np.asarray(src_keypoints, dtype=f)
    tkp = np.asarray(tgt_keypoints, dtype=f)
    rot = np.asarray(rotation_ab, dtype=f)
    tra = np.asarray(translation_ab, dtype=f)
    sknn = np.asarray(src_keypoints_knn, dtype=f)
    tknn = np.asarray(tgt_keypoints_knn, dtype=f)

    identh = np.eye(128, dtype=np.float16)
    kp_lhsT = np.zeros((4, 2 * 3), dtype=f)
    kp_rhs = np.zeros((4, 2 * KP), dtype=f)
    tgt_kp = np.zeros((3, 2 * KP), dtype=f)
    knn_src = np.zeros((128, 2 * 192), dtype=f)
    knn_tgt = np.zeros((128, 2 * 192), dtype=f)
    for b in range(B):
        kp_lhsT[0:3, b * 3:(b + 1) * 3] = rot[b].T
        kp_lhsT[3, b * 3:(b + 1) * 3] = tra[b]
        kp_rhs[0:3, b * KP:(b + 1) * KP] = skp[b]
        kp_rhs[3, b * KP:(b + 1) * KP] = 1.0
        tgt_kp[:, b * KP:(b + 1) * KP] = tkp[b]
        knn_src[:, b * 192:(b + 1) * 192] = sknn[b].reshape(128, 192)
        knn_tgt[:, b * 192:(b + 1) * 192] = tknn[b].reshape(128, 192)

    # t-side (tgt, M axis): 2-way bf16 split of -2*t, plus ||t||^2 in fp32
    tAs, nts = [], []
    for b in range(B):
        t2 = (-2.0 * tg[b]).astype(f)                       # [3, M]
        th = t2.astype(BF16NP)
        tl = (t2 - th.astype(f)).astype(BF16NP)
        tA = np.empty((K11, M), dtype=BF16NP)
        tA[0:3] = th
        tA[3:6] = th
        tA[6:9] = tl
        tA[9:11] = BF16NP(1.0)
        tAs.append(tA)
        nt = (tg[b].astype(np.float64) ** 2).sum(axis=0).astype(f)   # [M]
        nts.append(np.ascontiguousarray(nt.reshape(MI, 128).T))      # [128, MI]

    shared = {
        "identh": identh, "kp_lhsT": kp_lhsT, "kp_rhs": kp_rhs,
        "tgt_kp": tgt_kp, "knn_src": knn_src, "knn_tgt": knn_tgt,
    }
    in_maps = []
    for ci in range(NCORES):
        b, j = divmod(ci, NSHARDS)
        s = st[b][:, j * CHUNK:(j + 1) * CHUNK]             # [3, CHUNK]
        sh = s.astype(BF16NP)
        sl = (s - sh.astype(f)).astype(BF16NP)
        ns64 = (s.astype(np.float64) ** 2).sum(axis=0)      # [CHUNK]
        nsh = ns64.astype(BF16NP)
        nsl = (ns64 - nsh.astype(np.float64)).astype(BF16NP)
        sA = np.empty((K11, CHUNK), dtype=BF16NP)
        sA[0:3] = sh
        sA[3:6] = sl
        sA[6:9] = sh
        sA[9] = nsh
        sA[10] = nsl
        m = dict(shared)
        m["tA"] = tAs[b]
        m["sA"] = sA
        m["nt"] = nts[b]
        in_maps.append(m)
    return in_maps


def _huber(x, c):
    return np.where(x < c, 0.5 * x * x, c * x - 0.5 * c * c)


def _postprocess(results):
    c = np.float64(MARGIN)
    loss1 = np.float64(0.0)
    loss2 = np.float64(0.0)
    for b in range(B):
        rowmins = []
        for j in range(NSHARDS):
            r = results[b * NSHARDS + j]
            # raw partials [p, slot*CHUNK]: final min over (p, slot) here
            colmin = (np.asarray(r["colmin"], dtype=np.float64)
                      .reshape(128, QG, CHUNK).min(axis=(0, 1)))
            loss1 += _huber(colmin, c).sum()
            rowmins.append(np.asarray(r["rowmin"], dtype=np.float64).T.ravel())
        rm = np.minimum.reduce(rowmins)
        loss2 += _huber(rm, c).sum()
    gal = loss1 + loss2

    misc = np.asarray(results[0]["misc"], dtype=np.float64)
    kp_loss = (misc[0:3, 0].sum() + misc[0:3, 1].sum()) / B
    knn_loss = (misc[:, 2].sum() + misc[:, 3].sum()) / (B * KNN)
    ncl = knn_loss + kp_loss
    return np.float32(ncl), np.float32(gal)


def run_device(in_maps, **kw):
    nc = _get_nc()
    return run_bass_kernel_spmd(nc, in_maps, list(range(NCORES)), **kw)


def kernel(src_keypoints, tgt_keypoints, rotation_ab, translation_ab,
           src_keypoints_knn, tgt_keypoints_knn, k, src_transformed, tgt,
           **_unused):
    in_maps = _prepare_in_maps(src_keypoints, tgt_keypoints, rotation_ab,
                               translation_ab, src_keypoints_knn,
                               tgt_keypoints_knn, src_transformed, tgt)
    res = run_device(in_maps)
    return _postprocess(res.results)
